# revision 1
# baseline (speedup 1.0000x reference)
"""Trainium2 Bass kernel for nn_AffineMaskGenerator.

For each pixel p with color x (3-vec from `images`) and shifted color y
(3-vec from `shifted_images`), and each class k:

    log_mask[b, k, h, w] = -||W_k @ x + b_k - y||^2 / (2 sigma^2)

Strategy (pure data parallel over batch, 4 images per NeuronCore):
  - Fold the affine map into one matmul: with s = 1/(sqrt(2)*sigma),
    diff = s*(W_k x - y) + s*b_k is linear in (x, y); the bias enters
    later through the Square activation's per-partition bias.
    MM1: lhsT [30, 120] x rhs [30, 512] -> PSUM [120, 512]; the 120
    rows are 5 pixel-groups x (8 classes x 3 channels) = 5 px/PE-cycle.
    Four MM1s run concurrently in disjoint PE row groups 0/32/64/96
    (tile_position packing, 4 super-tiles per "quad").
  - Square via ScalarE activation(Square, bias=s*b) into bf16; ~1.5 of
    12 square tiles per quad go to VectorE (tensor_scalar add + bf16
    tensor_mul) to balance the engines.  DVE cannot read PSUM twice,
    so plain tensor_mul on PSUM is unavailable.
  - MM2: lhsT [120, 40] of -1 entries sums squares over channels ->
    [40, 512] final values in PSUM (5 groups x 8 classes).  Chunks
    (i, i+3) pack into one PSUM bank at partition offsets 0/64
    (tile_position) so a single DVE copy evacuates both, and each obuf
    half is a contiguous 1536-px run per (group, class) -> one 3-dim
    store DMA per half.
  - Matmuls run in bf16 (this walrus build rejects f32/f32r matmuls;
    PE is also clamped to ~1.2 GHz here, so tile_position concurrency
    is the only matmul-throughput lever).  Inputs are pre-cast to bf16
    on the host and concatenated to one [BL, 6, H, W] tensor: one
    3-dim SWDGE DMA loads a whole super-tile band set.
  - Pixel groups are contiguous 3072-px bands inside each 15360-px
    super-tile; the image tail is covered by an overlapping
    (idempotent) extra tile per image.
"""

import ml_dtypes
import numpy as np

import concourse.bass as bass
import concourse.mybir as mybir
import concourse.tile as tile
from concourse.tile import ScopedClock
from concourse import bass_utils

F32 = mybir.dt.float32
BF16 = mybir.dt.bfloat16

B, C, H, Wd = 32, 3, 512, 512
K = 8
NCORES = 8
BL = B // NCORES            # images per core
PLANE = H * Wd              # 262144 pixels per channel plane

N = 512                     # pixels per chunk (one PSUM bank of f32)
G = 5                       # pixel groups per tile (5 px / PE cycle)
TPS = 6                     # chunks per super-tile
BAND = TPS * N              # 3072 px: one group's contiguous band
SPX = G * BAND              # 15360 pixels per super-tile
NSUP = PLANE // SPX         # 17 full super-tiles per image
OV_BASE = PLANE - G * N     # overlap tile covers the image tail

_patched = False


MAX_WAITS = 1   # this walrus build rejects instructions with more sync waits


def _split_excess_waits(nc):
    """Walrus 'Too many sync wait commands': any instruction carrying
    more than MAX_WAITS sem waits gets the excess moved onto fresh NoOps
    inserted just before it on the same engine (engines execute their
    instruction stream in block order, so semantics are unchanged)."""
    import bass_rust
    counter = [0]
    for f in nc.m.functions:
        for bb in f.blocks:
            new_insts = []
            for inst in bb.instructions:
                si = inst.sync_info
                waits = list(si.on_wait or []) if si is not None else []
                if len(waits) > MAX_WAITS:
                    rest = waits[:-MAX_WAITS]
                    si.on_wait = waits[-MAX_WAITS:]
                    while rest:
                        counter[0] += 1
                        nop = bass_rust.InstNoOp(
                            name=f"waitsplit_{counter[0]}", ins=[], outs=[])
                        nop.engine = inst.engine
                        nop.sync_info = mybir.SyncInfo(
                            on_wait=rest[:MAX_WAITS], on_update=[])
                        rest = rest[MAX_WAITS:]
                        new_insts.append(nop)
                new_insts.append(inst)
            bb.instructions = new_insts


def _patch_tile_drain():
    """Rebuild the kernel-tail drain with split waits + run the global
    excess-wait splitter after Tile lowering."""
    global _patched
    if _patched:
        return
    _patched = True

    def _drain_and_barrier(self, tick_clock, wait_clock):
        drain_inst = self.nc.sync.drain()
        wait_clock.add_sem_waits(
            drain_inst.ins, ScopedClock({None: tick_clock.global_clock})
        )
        si = drain_inst.ins.sync_info
        waits = list(si.on_wait or []) if si is not None else []
        if len(waits) > 1:
            si.on_wait = waits[:1]
            for w in waits[1:]:
                d2 = self.nc.sync.drain()
                d2.ins.sync_info = mybir.SyncInfo(on_wait=[w], on_update=[])
        self.nc.all_engine_barrier()
        popped = self.nc._tile_sem_poison_stack.pop()
        assert popped is self._sem_poison
        self.nc.clear_and_free_semaphores(list(self.sems.allocated().values()))
        self.nc.all_engine_barrier()
        _split_excess_waits(self.nc)

    tile.TileContext._drain_and_barrier = _drain_and_barrier


def _host_weights(Wm, bm, sigma):
    """w1 [31, 120]: row 5c+g = x_c of group g, 15+5o+g = y_o of group g,
    30 = ones; col m = 24g+3k+o.  w2 [120, 40]: channel-sum, col 8g+k."""
    s = 1.0 / (np.sqrt(2.0) * float(sigma))
    w1 = np.zeros((30, 120), np.float32)
    w2 = np.zeros((120, 40), np.float32)
    bias = np.zeros((120, 1), np.float32)
    for g in range(G):
        for k in range(K):
            for o in range(C):
                m = 24 * g + 3 * k + o
                for c in range(C):
                    w1[5 * c + g, m] = s * Wm[k, o, c]
                w1[15 + 5 * o + g, m] = -s
                bias[m, 0] = s * bm[k, o]
                w2[m, 8 * g + k] = -1.0
    return w1, w2, bias


def build_nc():
    _patch_tile_drain()
    nc = bass.Bass("TRN2", target_bir_lowering=False, debug=False)
    # xy: host-side concat of images & shifted along channels, pre-cast
    # to bf16 (halves input DMA bytes, avoids the slow SWDGE cast path)
    xy = nc.dram_tensor("xy", [BL, 2 * C, H, Wd], BF16, kind="ExternalInput")
    w1 = nc.dram_tensor("w1", [30, 120], BF16, kind="ExternalInput")
    w2 = nc.dram_tensor("w2", [120, 40], BF16, kind="ExternalInput")
    bias = nc.dram_tensor("bias", [120, 1], F32, kind="ExternalInput")
    out = nc.dram_tensor("out", [BL, K, H, Wd], F32, kind="ExternalOutput")

    from contextlib import ExitStack
    with tile.TileContext(nc, pool_alloc_mode="queue") as tc, ExitStack() as ctx:
        singles = ctx.enter_context(tc.tile_pool(name="singles", bufs=1))
        chan_pool = ctx.enter_context(tc.tile_pool(name="chan", bufs=6))
        sq_pool = ctx.enter_context(tc.tile_pool(name="sq", bufs=26))
        obuf_pool = ctx.enter_context(tc.tile_pool(name="obuf", bufs=6))
        tmp_pool = ctx.enter_context(tc.tile_pool(name="tmp", bufs=4))
        pd_pool = ctx.enter_context(tc.tile_pool(name="pd", bufs=3, space="PSUM"))
        po_pool = ctx.enter_context(tc.tile_pool(name="po", bufs=2, space="PSUM"))

        # w1 replicated at partition bands 0/32/64/96 so four MM1s run
        # concurrently in disjoint PE row groups (tile_position packing)
        w1_t = singles.tile([126, 120], BF16)
        for r in range(4):
            nc.gpsimd.dma_start(out=w1_t[32 * r:32 * r + 30, :], in_=w1.ap())
        w2_t = singles.tile([120, 40], BF16)
        nc.gpsimd.dma_start(out=w2_t[:, :], in_=w2.ap())
        # per-partition bias s*b[k,o]: applied inside the Square activation
        # (ScalarE) / via tensor_scalar add (VectorE path)
        bias_t = singles.tile([120, 1], F32)
        nc.gpsimd.dma_start(out=bias_t[:, :], in_=bias.ap())

        def mm1(chan, band, pd, t, j):
            """Chunk j of `chan` band -> pd column t.  Bands use disjoint
            PE row groups, so the four mm1s execute concurrently."""
            nc.tensor.matmul(
                pd[:, bass.ts(t, N)], w1_t[band:band + 30, :],
                chan[band + 0:band + 30, bass.ts(j, N)],
                start=True, stop=True, tile_position=(band, 0))

        def square(pd, n_tiles):
            sq = sq_pool.tile([120, 2 * N], BF16, tag="sq")
            nc.scalar.activation(
                sq[:, 0:n_tiles * N], pd[:, 0:n_tiles * N],
                mybir.ActivationFunctionType.Square,
                bias=bias_t[:, 0:1], scale=1.0)
            return sq

        def square_dve(pd):
            """Square via VectorE: PSUM->SBUF bf16 copy (1x) + bf16
            tensor_mul (2x).  Less efficient than ScalarE but runs on
            the otherwise under-used DVE -- used to offload ScalarE."""
            tmp = tmp_pool.tile([120, 2 * N], BF16, tag="tmp")
            nc.vector.tensor_scalar_add(tmp[:, :], pd[:, 0:2 * N],
                                        bias_t[:, 0:1])
            sq = sq_pool.tile([120, 2 * N], BF16, tag="sq")
            nc.vector.tensor_mul(sq[:, :], tmp[:, :], tmp[:, :])
            return sq

        def emit_pair(sq_of, i, obuf):
            """MM2 for chunks (i, i+3) -> one po bank at partition
            offsets 0/64 -> DVE copy into obuf column block i.  With
            this pairing obuf[0:40] holds chunks 0..2 and obuf[64:104]
            chunks 3..5 -- each a contiguous 1536-px range per (g, k),
            so one 3-dim DMA stores each half."""
            po = po_pool.tile([104, N], F32, tag="po")
            for t, j in enumerate((i, i + 3)):
                sq, col = sq_of(j)
                nc.tensor.matmul(
                    po[64 * t:64 * t + 40, :], w2_t[:, :],
                    sq[:, bass.ts(col, N)],
                    start=True, stop=True, tile_position=(0, 64 * t))
            nc.vector.tensor_copy(obuf[:, bass.ts(i, N)], po[0:104, :])

        def load_chan(chan, band, s):
            """Load super-tile s (global index) into `chan` band 0/64."""
            img, S = divmod(s, NSUP)
            ib = img * 2 * C * PLANE
            px0 = S * SPX
            src_xy = bass.AP(xy, ib + px0,
                             [[PLANE, 2 * C], [BAND, G], [1, BAND]])
            nc.gpsimd.dma_start(out=chan[band + 0:band + 30], in_=src_xy)

        def store_out(obuf, s):
            img, S = divmod(s, NSUP)
            ob_base = img * K * PLANE
            px0 = S * SPX
            for half in range(2):
                src = bass.AP(
                    obuf.tensor, obuf.offset + half * 64 * (3 * N),
                    [[3 * N, 40], [1, 3 * N]])
                dst = bass.AP(out, ob_base + px0 + half * (3 * N),
                              [[BAND, G], [PLANE, K], [1, 3 * N]])
                nc.sync.dma_start(out=dst, in_=src)

        def emit_mm2_store(sqs, s0):
            for b in range(4):
                half, bi = b // 2, b % 2
                obuf = obuf_pool.tile([104, 3 * N], F32, tag="obuf")

                def sq_of(j, half=half, bi=bi):
                    return (sqs[(half, j)], bi)
                for i in range(3):
                    emit_pair(sq_of, i, obuf)
                store_out(obuf, s0 + b)

        NSUP_CORE = BL * NSUP          # 68 super-tiles per core
        prev = None
        for q in range(NSUP_CORE // 4):
            chan = chan_pool.tile([127, TPS * N], BF16, tag="chan")
            for r in range(4):
                load_chan(chan, 32 * r, 4 * q + r)

            sqs = {}
            for j in range(TPS):
                pd01 = pd_pool.tile([120, 2 * N], F32, tag="pd")
                mm1(chan, 0, pd01, 0, j)
                mm1(chan, 32, pd01, 1, j)
                sqs[(0, j)] = square(pd01, 2)
                pd23 = pd_pool.tile([120, 2 * N], F32, tag="pd")
                mm1(chan, 64, pd23, 0, j)
                mm1(chan, 96, pd23, 1, j)
                if j == 2 and q % 2 == 1:
                    sqs[(1, j)] = square_dve(pd23)
                else:
                    sqs[(1, j)] = square(pd23, 2)

            # software-pipeline: previous quad's MM2/store after this
            # quad's MM1s
            if prev is not None:
                emit_mm2_store(*prev)
            prev = (sqs, 4 * q)
        emit_mm2_store(*prev)

        # image-tail overlap tiles (re-compute 1536 px each; idempotent).
        for img in range(BL):
            ib = img * 2 * C * PLANE
            ob_base = img * K * PLANE
            chan = chan_pool.tile([95, TPS * N], BF16, tag="chan")
            src_xy = bass.AP(xy, ib + OV_BASE, [[PLANE, 2 * C], [N, G], [1, N]])
            nc.gpsimd.dma_start(out=chan[0:30, 0:N], in_=src_xy)
            obuf = obuf_pool.tile([104, 3 * N], F32, tag="obuf")
            pd = pd_pool.tile([120, 2 * N], F32, tag="pd")
            mm1(chan, 0, pd, 0, 0)
            sq_ov = square(pd, 1)
            po = po_pool.tile([104, N], F32, tag="po")
            nc.tensor.matmul(po[0:40, :], w2_t[:, :], sq_ov[:, bass.ts(0, N)],
                             start=True, stop=True, tile_position=(0, 0))
            nc.vector.tensor_copy(obuf[:, bass.ts(0, N)], po[0:104, :])
            dst = bass.AP(out, ob_base + OV_BASE, [[N, G], [PLANE, K], [1, N]])
            src = bass.AP(obuf.tensor, obuf.offset, [[3 * N, 40], [1, N]])
            nc.sync.dma_start(out=dst, in_=src)

    return nc


def _run(in_maps, trace=False, tmpdir=None):
    nc = build_nc()
    if trace:
        # inject the NTFF profile hook (normally absent in this image)
        import sys, types
        from trn_agent_boot.trn_boot import _ntff_profile_via_ctypes
        hook = _ntff_profile_via_ctypes('/opt/axon/libaxon_pjrt.so')
        m = types.ModuleType("antenv.axon_hooks")
        m.get_axon_ntff_profile_hook = lambda: hook
        m.set_axon_ntff_profile_hook = lambda h: None
        sys.modules["antenv.axon_hooks"] = m
        bass_utils.upload_artifacts = lambda d: d
    return bass_utils.run_bass_kernel_spmd(
        nc, in_maps, core_ids=list(range(NCORES)), trace=trace, tmpdir=tmpdir)


def make_in_maps(images, shifted_images, W, b, sigma):
    w1, w2, bias = _host_weights(np.asarray(W), np.asarray(b), sigma)
    w1 = w1.astype(ml_dtypes.bfloat16)
    w2 = w2.astype(ml_dtypes.bfloat16)
    xy = np.concatenate(
        [np.asarray(images, np.float32), np.asarray(shifted_images, np.float32)],
        axis=1).astype(ml_dtypes.bfloat16)
    in_maps = []
    for i in range(NCORES):
        in_maps.append({
            "xy": np.ascontiguousarray(xy[BL * i:BL * (i + 1)]),
            "w1": w1, "w2": w2, "bias": bias,
        })
    return in_maps


def kernel(images, shifted_images, W, b, sigma):
    in_maps = make_in_maps(images, shifted_images, W, b, sigma)
    res = _run(in_maps, trace=False)
    return np.concatenate([res.results[i]["out"] for i in range(NCORES)], axis=0)



# revision 7
# speedup vs baseline: 1.3057x; 1.3057x over previous
"""Trainium2 Bass kernel for nn_AffineMaskGenerator.

For each pixel p with color x (3-vec from `images`) and shifted color y
(3-vec from `shifted_images`), and each class k:

    log_mask[b, k, h, w] = -||W_k @ x + b_k - y||^2 / (2 sigma^2)

Strategy (pure data parallel over batch, 4 images per NeuronCore):
  - Fold the affine map into one matmul: with s = 1/(sqrt(2)*sigma),
    diff = s*(W_k x - y) + s*b_k is linear in (x, y); the bias enters
    later through the Square activation's per-partition bias.
    MM1: lhsT [30, 120] x rhs [30, 512] -> PSUM [120, 512]; the 120
    rows are 5 pixel-groups x (8 classes x 3 channels) = 5 px/PE-cycle.
    Four MM1s run concurrently in disjoint PE row groups 0/32/64/96
    (tile_position packing, 4 super-tiles per "quad").
  - Square via ScalarE activation(Square, bias=s*b) into bf16; ~1.5 of
    12 square tiles per quad go to VectorE (tensor_scalar add + bf16
    tensor_mul) to balance the engines.  DVE cannot read PSUM twice,
    so plain tensor_mul on PSUM is unavailable.
  - MM2: lhsT [120, 40] of -1 entries sums squares over channels ->
    [40, 512] final values in PSUM (5 groups x 8 classes).  Chunks
    (i, i+3) pack into one PSUM bank at partition offsets 0/64
    (tile_position) so a single DVE copy evacuates both, and each obuf
    half is a contiguous 1536-px run per (group, class) -> one 3-dim
    store DMA per half.
  - Matmuls run in bf16 (this walrus build rejects f32/f32r matmuls;
    PE is also clamped to ~1.2 GHz here, so tile_position concurrency
    is the only matmul-throughput lever).  Inputs are pre-cast to bf16
    on the host and concatenated to one [BL, 6, H, W] tensor: one
    3-dim SWDGE DMA loads a whole super-tile band set.
  - Pixel groups are contiguous 3072-px bands inside each 15360-px
    super-tile; the image tail is covered by an overlapping
    (idempotent) extra tile per image.
"""

import ml_dtypes
import numpy as np

import concourse.bass as bass
import concourse.mybir as mybir
import concourse.tile as tile
from concourse.tile import ScopedClock
from concourse import bass_utils

F32 = mybir.dt.float32
BF16 = mybir.dt.bfloat16

B, C, H, Wd = 32, 3, 512, 512
K = 8
NCORES = 8
BL = B // NCORES            # images per core
PLANE = H * Wd              # 262144 pixels per channel plane

N = 512                     # pixels per chunk (one PSUM bank of f32)
G = 5                       # pixel groups per tile (5 px / PE cycle)
TPS = 6                     # chunks per super-tile
BAND = TPS * N              # 3072 px: one group's contiguous band
SPX = G * BAND              # 15360 pixels per super-tile
NSUP = PLANE // SPX         # 17 full super-tiles per image
OV_BASE = PLANE - G * N     # overlap tile covers the image tail

_patched = False


MAX_WAITS = 1   # this walrus build rejects instructions with more sync waits


def _split_excess_waits(nc):
    """Walrus 'Too many sync wait commands': any instruction carrying
    more than MAX_WAITS sem waits gets the excess moved onto fresh NoOps
    inserted just before it on the same engine (engines execute their
    instruction stream in block order, so semantics are unchanged)."""
    import bass_rust
    counter = [0]
    for f in nc.m.functions:
        for bb in f.blocks:
            new_insts = []
            for inst in bb.instructions:
                si = inst.sync_info
                waits = list(si.on_wait or []) if si is not None else []
                if len(waits) > MAX_WAITS:
                    rest = waits[:-MAX_WAITS]
                    si.on_wait = waits[-MAX_WAITS:]
                    while rest:
                        counter[0] += 1
                        nop = bass_rust.InstNoOp(
                            name=f"waitsplit_{counter[0]}", ins=[], outs=[])
                        nop.engine = inst.engine
                        nop.sync_info = mybir.SyncInfo(
                            on_wait=rest[:MAX_WAITS], on_update=[])
                        rest = rest[MAX_WAITS:]
                        new_insts.append(nop)
                new_insts.append(inst)
            bb.instructions = new_insts


def _patch_tile_drain():
    """Rebuild the kernel-tail drain with split waits + run the global
    excess-wait splitter after Tile lowering."""
    global _patched
    if _patched:
        return
    _patched = True

    def _drain_and_barrier(self, tick_clock, wait_clock):
        drain_inst = self.nc.sync.drain()
        wait_clock.add_sem_waits(
            drain_inst.ins, ScopedClock({None: tick_clock.global_clock})
        )
        si = drain_inst.ins.sync_info
        waits = list(si.on_wait or []) if si is not None else []
        if len(waits) > 1:
            si.on_wait = waits[:1]
            for w in waits[1:]:
                d2 = self.nc.sync.drain()
                d2.ins.sync_info = mybir.SyncInfo(on_wait=[w], on_update=[])
        self.nc.all_engine_barrier()
        popped = self.nc._tile_sem_poison_stack.pop()
        assert popped is self._sem_poison
        self.nc.clear_and_free_semaphores(list(self.sems.allocated().values()))
        self.nc.all_engine_barrier()
        _split_excess_waits(self.nc)

    tile.TileContext._drain_and_barrier = _drain_and_barrier


def _host_weights(Wm, bm, sigma):
    """w1 [31, 120]: row 5c+g = x_c of group g, 15+5o+g = y_o of group g,
    30 = ones; col m = 24g+3k+o.  w2 [120, 40]: channel-sum, col 8g+k."""
    s = 1.0 / (np.sqrt(2.0) * float(sigma))
    w1 = np.zeros((30, 120), np.float32)
    w2 = np.zeros((120, 40), np.float32)
    bias = np.zeros((120, 1), np.float32)
    for g in range(G):
        for k in range(K):
            for o in range(C):
                m = 24 * g + 3 * k + o
                for c in range(C):
                    w1[5 * c + g, m] = s * Wm[k, o, c]
                w1[15 + 5 * o + g, m] = -s
                bias[m, 0] = s * bm[k, o]
                w2[m, 8 * g + k] = -1.0
    return w1, w2, bias


def build_nc():
    _patch_tile_drain()
    nc = bass.Bass("TRN2", target_bir_lowering=False, debug=False)
    # xy: host-side concat of images & shifted along channels, pre-cast
    # to bf16 (halves input DMA bytes, avoids the slow SWDGE cast path)
    xy = nc.dram_tensor("xy", [BL, 2 * C, H, Wd], BF16, kind="ExternalInput")
    w1 = nc.dram_tensor("w1", [30, 120], BF16, kind="ExternalInput")
    w2 = nc.dram_tensor("w2", [120, 40], BF16, kind="ExternalInput")
    bias = nc.dram_tensor("bias", [120, 1], F32, kind="ExternalInput")
    # Output in bf16 (halves store bytes; rel-err budget is 2e-2) and in a
    # HW-friendly scrambled layout: each half-obuf stores as one fully
    # contiguous 40x1536 run, so the store AP's outer dim is 40 partition
    # rows -> HWDGE sprays across all 16 SDMA engines instead of 5.  The
    # host unscrambles (postprocess).
    out = nc.dram_tensor("out", [BL, NSUP, 2, 40, 3 * N], BF16,
                         kind="ExternalOutput")
    out_tail = nc.dram_tensor("out_tail", [BL, 40, N], BF16,
                              kind="ExternalOutput")

    from contextlib import ExitStack
    with tile.TileContext(nc, pool_alloc_mode="queue") as tc, ExitStack() as ctx:
        singles = ctx.enter_context(tc.tile_pool(name="singles", bufs=1))
        chan_pool = ctx.enter_context(tc.tile_pool(name="chan", bufs=6))
        sq_pool = ctx.enter_context(tc.tile_pool(name="sq", bufs=26))
        obuf_pool = ctx.enter_context(tc.tile_pool(name="obuf", bufs=6))
        tmp_pool = ctx.enter_context(tc.tile_pool(name="tmp", bufs=4))
        pd_pool = ctx.enter_context(tc.tile_pool(name="pd", bufs=3, space="PSUM"))
        po_pool = ctx.enter_context(tc.tile_pool(name="po", bufs=2, space="PSUM"))

        # w1 replicated at partition bands 0/32/64/96 so four MM1s run
        # concurrently in disjoint PE row groups (tile_position packing)
        w1_t = singles.tile([126, 120], BF16)
        for r in range(4):
            nc.gpsimd.dma_start(out=w1_t[32 * r:32 * r + 30, :], in_=w1.ap())
        w2_t = singles.tile([120, 40], BF16)
        nc.gpsimd.dma_start(out=w2_t[:, :], in_=w2.ap())
        # per-partition bias s*b[k,o]: applied inside the Square activation
        # (ScalarE) / via tensor_scalar add (VectorE path)
        bias_t = singles.tile([120, 1], F32)
        nc.gpsimd.dma_start(out=bias_t[:, :], in_=bias.ap())

        def mm1(chan, band, pd, t, j):
            """Chunk j of `chan` band -> pd column t.  Bands use disjoint
            PE row groups, so the four mm1s execute concurrently."""
            nc.tensor.matmul(
                pd[:, bass.ts(t, N)], w1_t[band:band + 30, :],
                chan[band + 0:band + 30, bass.ts(j, N)],
                start=True, stop=True, tile_position=(band, 0))

        def square(pd, n_tiles):
            sq = sq_pool.tile([120, 2 * N], BF16, tag="sq")
            nc.scalar.activation(
                sq[:, 0:n_tiles * N], pd[:, 0:n_tiles * N],
                mybir.ActivationFunctionType.Square,
                bias=bias_t[:, 0:1], scale=1.0)
            return sq

        def square_dve(pd):
            """Square via VectorE: PSUM->SBUF bf16 copy (1x) + bf16
            tensor_mul (2x).  Less efficient than ScalarE but runs on
            the otherwise under-used DVE -- used to offload ScalarE."""
            tmp = tmp_pool.tile([120, 2 * N], BF16, tag="tmp")
            nc.vector.tensor_scalar_add(tmp[:, :], pd[:, 0:2 * N],
                                        bias_t[:, 0:1])
            sq = sq_pool.tile([120, 2 * N], BF16, tag="sq")
            nc.vector.tensor_mul(sq[:, :], tmp[:, :], tmp[:, :])
            return sq

        def emit_pair(sq_of, i, obuf):
            """MM2 for chunks (i, i+3) -> one po bank at partition
            offsets 0/64 -> DVE copy into obuf column block i.  With
            this pairing obuf[0:40] holds chunks 0..2 and obuf[64:104]
            chunks 3..5 -- each a contiguous 1536-px range per (g, k),
            so one 3-dim DMA stores each half."""
            po = po_pool.tile([104, N], F32, tag="po")
            for t, j in enumerate((i, i + 3)):
                sq, col = sq_of(j)
                nc.tensor.matmul(
                    po[64 * t:64 * t + 40, :], w2_t[:, :],
                    sq[:, bass.ts(col, N)],
                    start=True, stop=True, tile_position=(0, 64 * t))
            nc.vector.tensor_copy(obuf[:, bass.ts(i, N)], po[0:104, :])

        def load_chan(chan, band, s):
            """Load super-tile s (global index) into `chan` band 0/64."""
            img, S = divmod(s, NSUP)
            ib = img * 2 * C * PLANE
            px0 = S * SPX
            src_xy = bass.AP(xy, ib + px0,
                             [[PLANE, 2 * C], [BAND, G], [1, BAND]])
            nc.gpsimd.dma_start(out=chan[band + 0:band + 30], in_=src_xy)

        def store_out(obuf, s):
            img, S = divmod(s, NSUP)
            for half in range(2):
                src = bass.AP(
                    obuf.tensor, obuf.offset + half * 64 * (3 * N),
                    [[3 * N, 40], [1, 3 * N]])
                dst = bass.AP(
                    out, ((img * NSUP + S) * 2 + half) * 40 * (3 * N),
                    [[3 * N, 40], [1, 3 * N]])
                nc.sync.dma_start(out=dst, in_=src)

        def emit_mm2_store(sqs, s0):
            for b in range(4):
                half, bi = b // 2, b % 2
                obuf = obuf_pool.tile([104, 3 * N], BF16, tag="obuf")

                def sq_of(j, half=half, bi=bi):
                    return (sqs[(half, j)], bi)
                for i in range(3):
                    emit_pair(sq_of, i, obuf)
                store_out(obuf, s0 + b)

        NSUP_CORE = BL * NSUP          # 68 super-tiles per core
        prev = None
        for q in range(NSUP_CORE // 4):
            chan = chan_pool.tile([127, TPS * N], BF16, tag="chan")
            for r in range(4):
                load_chan(chan, 32 * r, 4 * q + r)

            sqs = {}
            for j in range(TPS):
                pd01 = pd_pool.tile([120, 2 * N], F32, tag="pd")
                mm1(chan, 0, pd01, 0, j)
                mm1(chan, 32, pd01, 1, j)
                sqs[(0, j)] = square(pd01, 2)
                pd23 = pd_pool.tile([120, 2 * N], F32, tag="pd")
                mm1(chan, 64, pd23, 0, j)
                mm1(chan, 96, pd23, 1, j)
                if j == 2 or (j == 5 and q % 2 == 1):
                    sqs[(1, j)] = square_dve(pd23)
                else:
                    sqs[(1, j)] = square(pd23, 2)

            # software-pipeline: previous quad's MM2/store after this
            # quad's MM1s
            if prev is not None:
                emit_mm2_store(*prev)
            prev = (sqs, 4 * q)
        emit_mm2_store(*prev)

        # image-tail overlap tiles (re-compute 1536 px each; idempotent).
        for img in range(BL):
            ib = img * 2 * C * PLANE
            chan = chan_pool.tile([95, TPS * N], BF16, tag="chan")
            src_xy = bass.AP(xy, ib + OV_BASE, [[PLANE, 2 * C], [N, G], [1, N]])
            nc.gpsimd.dma_start(out=chan[0:30, 0:N], in_=src_xy)
            obuf = obuf_pool.tile([104, 3 * N], BF16, tag="obuf")
            pd = pd_pool.tile([120, 2 * N], F32, tag="pd")
            mm1(chan, 0, pd, 0, 0)
            sq_ov = square(pd, 1)
            po = po_pool.tile([104, N], F32, tag="po")
            nc.tensor.matmul(po[0:40, :], w2_t[:, :], sq_ov[:, bass.ts(0, N)],
                             start=True, stop=True, tile_position=(0, 0))
            nc.vector.tensor_copy(obuf[:, bass.ts(0, N)], po[0:104, :])
            dst = bass.AP(out_tail, img * 40 * N, [[N, 40], [1, N]])
            src = bass.AP(obuf.tensor, obuf.offset, [[3 * N, 40], [1, N]])
            nc.sync.dma_start(out=dst, in_=src)

    return nc


def _run(in_maps, trace=False, tmpdir=None):
    nc = build_nc()
    if trace:
        # inject the NTFF profile hook (normally absent in this image)
        import sys, types
        from trn_agent_boot.trn_boot import _ntff_profile_via_ctypes
        hook = _ntff_profile_via_ctypes('/opt/axon/libaxon_pjrt.so')
        m = types.ModuleType("antenv.axon_hooks")
        m.get_axon_ntff_profile_hook = lambda: hook
        m.set_axon_ntff_profile_hook = lambda h: None
        sys.modules["antenv.axon_hooks"] = m
        bass_utils.upload_artifacts = lambda d: d
    return bass_utils.run_bass_kernel_spmd(
        nc, in_maps, core_ids=list(range(NCORES)), trace=trace, tmpdir=tmpdir)


def make_in_maps(images, shifted_images, W, b, sigma):
    w1, w2, bias = _host_weights(np.asarray(W), np.asarray(b), sigma)
    w1 = w1.astype(ml_dtypes.bfloat16)
    w2 = w2.astype(ml_dtypes.bfloat16)
    xy = np.concatenate(
        [np.asarray(images, np.float32), np.asarray(shifted_images, np.float32)],
        axis=1).astype(ml_dtypes.bfloat16)
    in_maps = []
    for i in range(NCORES):
        in_maps.append({
            "xy": np.ascontiguousarray(xy[BL * i:BL * (i + 1)]),
            "w1": w1, "w2": w2, "bias": bias,
        })
    return in_maps


def postprocess(res):
    """Unscramble the HW output layout back to [B, K, H, W] float32.

    main[img, s, h, m=(8g+k), j] -> out[img, k, s*SPX + g*BAND + h*3N + j]
    tail[img, m=(8g+k), j]       -> out[img, k, OV_BASE + g*N + j]
    """
    outs = []
    for i in range(NCORES):
        main = np.asarray(res.results[i]["out"]).astype(np.float32)
        tail = np.asarray(res.results[i]["out_tail"]).astype(np.float32)
        main = main.reshape(BL, NSUP, 2, G, K, 3 * N)
        main = main.transpose(0, 4, 1, 3, 2, 5).reshape(BL, K, NSUP * SPX)
        full = np.empty((BL, K, PLANE), np.float32)
        full[:, :, :NSUP * SPX] = main
        tail = tail.reshape(BL, G, K, N).transpose(0, 2, 1, 3)
        full[:, :, OV_BASE:] = tail.reshape(BL, K, G * N)
        outs.append(full.reshape(BL, K, H, Wd))
    return np.concatenate(outs, axis=0)


def kernel(images, shifted_images, W, b, sigma):
    in_maps = make_in_maps(images, shifted_images, W, b, sigma)
    res = _run(in_maps, trace=False)
    return postprocess(res)

